# revision 71
# baseline (speedup 1.0000x reference)
"""Trainium2 Bass kernel for nn_ComplexAttention (B=4,H=8,T=2048,D=256).

Strategy
--------
* Shard the 32 (b,h) pairs across 8 NeuronCores, 4 per core (data parallel).
* Algebraic fusion removes two of the five projection GEMM groups:
    - scores only need Re(q conj(k))^T, so the k-projection folds into the
      q side:  u = xq @ A + i(...) with A = Wq^T Wk + ..., and
      scores^T = xkr^T-chunks against u directly (per-q constants from b_k
      are softmax-invariant and dropped; b_q becomes a u-bias row).
    - the o-projection folds into v: veff = xkv @ (W_o W_v)^T, so the
      attention context matmul directly produces the (unnormalized) output.
* All matmuls in fp16 (same PE rate as bf16, ~4x lower quantization noise).
* Softmax denominator off the PE: DVE strided tensor_reduce accumulates the
  16 exp chunks, one small all-ones matmul broadcasts the cross-partition
  sum, DVE fast-reciprocal normalizes at the PSUM drain.
* Single ACT function table (exp_and_others) for the whole kernel:
  the gate magnitude uses a two-segment alpha-max-beta-min approximation
  (DVE max/min + scalar_tensor_tensor, ~2.1% max err, ~3e-3 on the gated
  output), sigmoid(z) = 1/(1+exp(-z)) with the reciprocal on DVE. Zero
  ACT_TABLE_LOAD churn.
* Software-pipelined emission: produce(t+1) interleaves with consume(t);
  the sums finalization for tile t+1 is injected between consume(t)'s
  context matmuls and its gate matmuls so the PE never waits on the DVE
  reduce chain.
* veff uses the 3-mult Karatsuba complex product (6 matmuls per t-tile
  instead of 8); each tile is a deferred closure emitted in slots woven
  between the previous head's score/context chains, so the PSUM->DVE
  combine drains overlap independent PE work (dedicated 2-bank PSUM
  pool; m1 staged to SBUF so its bank is reused for m3).
* PE pre-warm: a run of junk matmuls on the ones tile issues during the
  initial DMA prologue so the HAM clock gate opens (1.2 -> 2.4 GHz)
  before the first real matmul.
* Drain ordering keeps the PE's in-order queue free of head-of-line
  blocking: each tile's mag chain (gate_act) is emitted right after its
  B-phase, Abs drains precede Identity drains on the in-order ACT
  engine, and all remaining context matmuls are emitted before the
  final gate matmuls.
* Outputs stored fp16 (halves the output DMA; ~1e-4 extra quantization).
"""

import numpy as np

B, H, T, D = 4, 8, 2048, 256
NCORES = 8
BH = B * H
BH_PER_CORE = BH // NCORES  # 4
P = 128
DC = D // P       # 2 chunks of the feature dim
QT = 512          # q-tile width (matmul free dim / PSUM bank)
NQT = T // QT     # 4 q-tiles
NKC = T // P      # 16 k-chunks
NTT = T // P      # 16 t-tiles for veff
EPS = 1e-8
SCALE = 1.0 / np.sqrt(D)
NWARM = 64        # PE pre-warm junk matmuls (covers the DMA prologue)
# two-segment alpha-max-beta-min magnitude approx: sqrt(a^2+b^2) ~=
# max(mx, AMAG*mx + BMAG*mn), max relative error ~2.1%
AMAG, BMAG = 0.898200, 0.486000

WNAMES = ["A", "Bm", "Bmn", "WerT", "WeiT", "WpT", "gwT"]
NW = len(WNAMES)
BNAMES = ["bur", "bui", "bor", "boi", "bgn"]
NB = len(BNAMES)

F16 = np.float16

_BUILT = None  # cache so repeated kernel() calls reuse the compiled module


def _emit_kernel(nc, tc, ctx, tens):
    from concourse import mybir

    f32 = mybir.dt.float32
    fp16 = mybir.dt.float16
    AF = mybir.ActivationFunctionType

    consts = ctx.enter_context(tc.tile_pool(name="consts", bufs=1))
    inpool = ctx.enter_context(tc.tile_pool(name="inpool", bufs=2))
    upool = ctx.enter_context(tc.tile_pool(name="upool", bufs=1))
    vpool = ctx.enter_context(tc.tile_pool(name="vpool", bufs=2))
    attnpool = ctx.enter_context(tc.tile_pool(name="attnpool", bufs=2))
    sumpool = ctx.enter_context(tc.tile_pool(name="sumpool", bufs=2))
    outppool = ctx.enter_context(tc.tile_pool(name="outppool", bufs=2))
    m2pool = ctx.enter_context(tc.tile_pool(name="m2pool", bufs=1))
    smallpool = ctx.enter_context(tc.tile_pool(name="smallpool", bufs=1))
    karpool = ctx.enter_context(tc.tile_pool(name="karpool", bufs=2))
    # ps_mm: produce score chains + sums + gate chains (exp/recip-drained).
    # ps_out: context chains (norm-drained, far apart).
    # ps_kar: one veff Karatsuba group: m1/m3 share a tile (m1 is staged to
    # SBUF before m3 overwrites), m2 the other.
    ps_mm = ctx.enter_context(tc.tile_pool(name="ps_mm", bufs=4, space="PSUM"))
    ps_out = ctx.enter_context(tc.tile_pool(name="ps_out", bufs=2, space="PSUM"))
    ps_kar = ctx.enter_context(tc.tile_pool(name="ps_kar", bufs=2, space="PSUM"))

    # ---- constants: A+Bm first (first matmul group's stationaries), rest
    # interleaved with the first input load ----
    bpack = consts.tile([P, NB * DC], f32, tag="bpack")
    bias = {name: bpack[:, i * DC:(i + 1) * DC] for i, name in enumerate(BNAMES)}

    wpack = consts.tile([P, DC, NW * D], fp16, tag="wpack")
    wp_dram = tens["wpack"]
    nc.sync.dma_start(wpack[:, :, :2 * D], wp_dram[:, :, :2 * D])  # A + Bm
    w = {name: wpack[:, :, i * D:(i + 1) * D] for i, name in enumerate(WNAMES)}

    nc.sync.dma_start(bpack[:], tens["bpack"][:])  # tiny; first u drain needs it

    def load_consts_rest():
        nc.sync.dma_start(wpack[:, :, 2 * D:], wp_dram[:, :, 2 * D:])

    ones_k = consts.tile([P, P], fp16, tag="ones_k")
    nc.vector.memset(ones_k[:], 1.0)

    # ---- PE pre-warm: junk matmuls (no DMA deps) keep the PE busy through
    # the input-DMA prologue so the HAM clock gate opens to 2.4 GHz before
    # the first real matmul. Uses a ps_kar bank (first real use is ~25us
    # in, long after these retire).
    jp = ps_kar.tile([P, QT], f32, tag="kar", name="warm")
    for i in range(NWARM):
        nc.tensor.matmul(jp[:, :P], ones_k[:], ones_k[:],
                         start=(i == 0), stop=(i == NWARM - 1),
                         skip_group_check=True)

    # ---- deferred side-work queue: veff Karatsuba groups are emitted one
    # at a time in "slots" between the score/context matmul chains of the
    # preceding head, so their PSUM->DVE combine drains always overlap
    # independent PE work and the in-order PE queue never stalls on a
    # bank WAR.
    side_q = []

    def side():
        if side_q:
            side_q.pop(0)()

    def load_inputs(bh, first=False):
        """DMA the 4 input tensors for one (b,h); xq in halves so the first
        u-projection matmuls can start as soon as possible."""
        with nc.named_scope(f"load{bh}"):
            # xq is dead after proj_u, long before the next load is emitted:
            # single-buffered (xkv stays double-buffered for the prefetch)
            xqr = inpool.tile([P, DC, T], fp16, tag="xqr", name="xqr", bufs=1)
            xqi = inpool.tile([P, DC, T], fp16, tag="xqi", name="xqi", bufs=1)
            xkr = inpool.tile([P, DC, T], fp16, tag="xkr", name="xkr")
            xki = inpool.tile([P, DC, T], fp16, tag="xki", name="xki")
            if first:
                # single sync ring in strict priority order matching
                # first-use time: the ring then runs at full HBM rate with
                # no competing queue stealing bandwidth from the critical
                # first u-proj tiles (proj_u consumes tt-outer)
                nc.sync.dma_start(xqr[:, :, :QT], tens["xq_r"][bh][:, :, :QT])
                nc.sync.dma_start(xqi[:, :, :QT], tens["xq_i"][bh][:, :, :QT])
                nc.sync.dma_start(xqr[:, :, QT:2 * QT],
                                  tens["xq_r"][bh][:, :, QT:2 * QT])
                nc.sync.dma_start(xqi[:, :, QT:2 * QT],
                                  tens["xq_i"][bh][:, :, QT:2 * QT])
                nc.sync.dma_start(xqr[:, :, 2 * QT:3 * QT],
                                  tens["xq_r"][bh][:, :, 2 * QT:3 * QT])
                nc.sync.dma_start(xqi[:, :, 2 * QT:3 * QT],
                                  tens["xq_i"][bh][:, :, 2 * QT:3 * QT])
                nc.sync.dma_start(xqr[:, :, 3 * QT:],
                                  tens["xq_r"][bh][:, :, 3 * QT:])
                nc.sync.dma_start(xqi[:, :, 3 * QT:],
                                  tens["xq_i"][bh][:, :, 3 * QT:])
                load_consts_rest()
            else:
                half = T // 2
                for t, name in ((xqr, "xq_r"), (xqi, "xq_i")):
                    nc.sync.dma_start(t[:, :, :half],
                                      tens[name][bh][:, :, :half])
                    nc.sync.dma_start(t[:, :, half:],
                                      tens[name][bh][:, :, half:])
            for t, name in ((xkr, "xkv_r"), (xki, "xkv_i")):
                nc.sync.dma_start(t[:], tens[name][bh][:])
        return {"xqr": xqr, "xqi": xqi, "xkr": xkr, "xki": xki}

    def proj_u(bh, ld):
        """u_r = xq_r@A + xq_i@Bm + bur ; u_i = xq_i@A - xq_r@Bm + bui.
        d-major output [P, DC, T] fp16 (weights stationary)."""
        xqr, xqi = ld["xqr"], ld["xqi"]
        with nc.named_scope(f"u{bh}"):
            u_r = upool.tile([P, DC, T], fp16, tag="u_r")
            u_i = upool.tile([P, DC, T], fp16, tag="u_i")
            # tt-outer so the matmul order matches the xq DMA arrival order
            # (the first q-tile's 16 matmuls need only xq[:, :, :QT])
            for tt in range(NQT):
                ts = slice(tt * QT, (tt + 1) * QT)
                for dst, s1, w1, s2, w2, b in (
                    (u_r, xqr, "A", xqi, "Bm", "bur"),
                    (u_i, xqi, "A", xqr, "Bmn", "bui"),
                ):
                    for co in range(DC):
                        ps = ps_mm.tile([P, QT], f32, tag="mm")
                        nc.tensor.matmul(ps, w[w1][:, 0, co * P:(co + 1) * P],
                                         s1[:, 0, ts], start=True, stop=False)
                        nc.tensor.matmul(ps, w[w1][:, 1, co * P:(co + 1) * P],
                                         s1[:, 1, ts], start=False, stop=False)
                        nc.tensor.matmul(ps, w[w2][:, 0, co * P:(co + 1) * P],
                                         s2[:, 0, ts], start=False, stop=False)
                        nc.tensor.matmul(ps, w[w2][:, 1, co * P:(co + 1) * P],
                                         s2[:, 1, ts], start=False, stop=True)
                        nc.scalar.activation(dst[:, co, ts], ps, AF.Identity,
                                             bias=bias[b][:, co:co + 1])
        return u_r, u_i

    def enqueue_veff(bh, ld):
        """veff = xkv @ (W_o W_v)^T in t-major [P, NTT, D] fp16, computed
        with the 3-mult Karatsuba complex product:
          m1 = xkr@WerT, m2 = xki@WeiT, m3 = (xkr+xki)@(WerT+WeiT)
          v_r = m1 - m2,  v_i = m3 - m1 - m2
        6 matmuls per t-tile instead of 8. Each t-tile is one closure on
        side_q; the DVE combines drain while slot-adjacent PE work runs.
        Tiles are allocated now so the caller can reference them."""
        v_r = vpool.tile([P, NTT, D], fp16, tag="v_r")
        v_i = vpool.tile([P, NTT, D], fp16, tag="v_i")
        xkr, xki = ld["xkr"], ld["xki"]

        def group(tt):
            tsl = slice(tt * P, (tt + 1) * P)
            with nc.named_scope(f"veff{bh}_{tt}"):
                xks = karpool.tile([P, DC, P], fp16, tag="xks", name="xks")
                nc.vector.tensor_add(xks[:], xkr[:, :, tsl], xki[:, :, tsl])
                pa = ps_kar.tile([P, QT], f32, tag="kar", name="pa")[:, :D]
                pb = ps_kar.tile([P, QT], f32, tag="kar", name="pb")[:, :D]
                for mm, x0, x1, wname in (
                        (pa, xkr[:, 0, tsl], xkr[:, 1, tsl], "WerT"),
                        (pb, xki[:, 0, tsl], xki[:, 1, tsl], "WeiT")):
                    nc.tensor.matmul(mm, x0, w[wname][:, 0, :],
                                     start=True, stop=False)
                    nc.tensor.matmul(mm, x1, w[wname][:, 1, :],
                                     start=False, stop=True)
                # DVE reads at most one PSUM input per op: stage m1 in SBUF,
                # then reuse its bank for m3
                s1 = karpool.tile([P, D], fp16, tag="s1", name="s1")
                nc.vector.tensor_copy(s1[:], pa)
                nc.tensor.matmul(pa, xks[:, 0, :], w["WpT"][:, 0, :],
                                 start=True, stop=False)
                nc.tensor.matmul(pa, xks[:, 1, :], w["WpT"][:, 1, :],
                                 start=False, stop=True)
                kt = karpool.tile([P, D], f32, tag="kt", name="kt")
                nc.vector.tensor_sub(v_r[:, tt, :], s1[:], pb)
                nc.vector.tensor_sub(kt[:], pa, s1[:])
                nc.vector.tensor_sub(v_i[:, tt, :], kt[:], pb)

        for tt in range(NTT):
            side_q.append(lambda tt=tt: group(tt))
        return v_r, v_i

    def produce(st, qt):
        """scoresT chunks -> exp -> DVE partial k-sums (fp16)."""
        u_r, u_i = st["u_r"], st["u_i"]
        xkr, xki = st["xkr"], st["xki"]
        qsl = slice(qt * QT, (qt + 1) * QT)
        with nc.named_scope(f"attn{st['bh']}_{qt}"):
            attn = attnpool.tile([P, NKC, QT], fp16, tag="attn")
            tree = sumpool.tile([P, 4, QT], fp16, tag="tree")
            for kc in range(NKC):
                ksl = slice(kc * P, (kc + 1) * P)
                sc = ps_mm.tile([P, QT], f32, tag="mm")
                nc.tensor.matmul(sc, xkr[:, 0, ksl], u_r[:, 0, qsl],
                                 start=True, stop=False)
                nc.tensor.matmul(sc, xkr[:, 1, ksl], u_r[:, 1, qsl],
                                 start=False, stop=False)
                nc.tensor.matmul(sc, xki[:, 0, ksl], u_i[:, 0, qsl],
                                 start=False, stop=False)
                nc.tensor.matmul(sc, xki[:, 1, ksl], u_i[:, 1, qsl],
                                 start=False, stop=True)
                nc.scalar.activation(attn[:, kc, :], sc, AF.Exp, scale=SCALE)
                # partial softmax sums: contiguous fp16 adds on DVE, early
                # chunks folded while later chunks' score matmuls run
                if kc == 7:
                    nc.vector.tensor_add(tree[:], attn[:, 0:4, :],
                                         attn[:, 4:8, :])
                elif kc == 11:
                    nc.vector.tensor_add(tree[:], tree[:], attn[:, 8:12, :])
                elif kc == NKC - 1:
                    nc.vector.tensor_add(tree[:], tree[:], attn[:, 12:16, :])
                if kc % 4 == 3:
                    side()  # slot for one deferred veff Karatsuba group
            for width in (2, 1):
                nc.vector.tensor_add(tree[:, 0:width, :], tree[:, 0:width, :],
                                     tree[:, width:2 * width, :])
        return {"attn": attn, "p16": tree[:, 0, :], "qt": qt, "bh": st["bh"],
                "st": st}

    def sums_mm(pr):
        """all-ones matmul broadcasts the cross-partition sum into PSUM.
        (A GpSimd partition_all_reduce was tried instead: correct, but its
        latency does not hide -- ~12us slower overall.)"""
        with nc.named_scope(f"sums{pr['bh']}_{pr['qt']}"):
            sums = ps_mm.tile([P, QT], f32, tag="mm", name="sums")
            nc.tensor.matmul(sums, ones_k[:], pr["p16"],
                             start=True, stop=True)
        pr["sums"] = sums

    def sums_recip(pr):
        """DVE fast reciprocal -> bc (per-q normalization, partition-bcast)."""
        # bc lives from consume(t-1) (where it is computed) into consume(t):
        # needs its own double buffer
        bc = smallpool.tile([P, QT], f32, tag="bc", name="bc", bufs=2)
        nc.vector.reciprocal_approx_fast(bc[:], pr["sums"][:])
        pr["bc"] = bc

    def sums_fin(pr):
        sums_mm(pr)
        sums_recip(pr)

    def consume_out(pr, then_pr=None, qoff=0, qw=QT):
        """B-phase: context/output matmuls -> normalize -> bias-add + square
        drains -> |out| abs tiles, for columns [qoff, qoff+qw). then_pr:
        the next tile's sums finalization, injected after the context
        matmuls (PE) / after the normalize muls (DVE). Tiles allocate on
        the first call and are shared by later column-slices (tail split)."""
        st, qt, bh = pr["st"], pr["qt"], pr["bh"]
        attn, bc = pr["attn"], pr["bc"]
        v_r, v_i = st["v_r"], st["v_i"]
        csl = slice(qoff, qoff + qw)
        with nc.named_scope(f"out{bh}_{qt}"):
            if "out_r" not in pr:
                pr["out_r"] = outppool.tile([P, DC, QT], fp16, tag="out_r",
                                            name="out_r")
                pr["out_i"] = outppool.tile([P, DC, QT], fp16, tag="out_i",
                                            name="out_i")
                pr["ar"] = m2pool.tile([P, DC, QT], fp16, tag="ar",
                                       name="ar", bufs=2)
                pr["ai"] = m2pool.tile([P, DC, QT], fp16, tag="ai",
                                       name="ai", bufs=2)
                pr["tmp"] = smallpool.tile([P, 2 * DC, QT], fp16, tag="tmp",
                                           name="tmp")
            out_r = pr["out_r"][:, :, csl]
            out_i = pr["out_i"][:, :, csl]
            ar = pr["ar"][:, :, csl]
            ai = pr["ai"][:, :, csl]
            tmp = pr["tmp"][:, :, csl]
            # all Abs drains first: the gate's mag chain depends only on
            # them, so it never queues behind the Identity drains on the
            # in-order ACT engine (matters most in the pipeline drain)
            for di, (av, vsrc, b) in enumerate(((ar, v_r, "bor"),
                                                (ai, v_i, "boi"))):
                for c in range(DC):
                    cps = ps_out.tile([P, QT], f32, tag="outp",
                                      name="outp")[:, :qw]
                    for kc in range(NKC):
                        nc.tensor.matmul(cps, vsrc[:, kc, c * P:(c + 1) * P],
                                         attn[:, kc, csl],
                                         start=(kc == 0), stop=(kc == NKC - 1),
                                         skip_group_check=True)
                    # normalize, then abs (for the magnitude approximation)
                    nc.vector.tensor_mul(tmp[:, di * DC + c, :], cps,
                                         bc[:, csl])
                    nc.scalar.activation(av[:, c, :], tmp[:, di * DC + c, :],
                                         AF.Abs, bias=bias[b][:, c:c + 1])
                    side()  # slot for one deferred veff Karatsuba group

            if then_pr is not None:
                sums_mm(then_pr)  # PE: right after the context matmuls
                sums_recip(then_pr)  # DVE: after the normalize muls

            # bias-add drains (Identity) after all Abs passes
            for di, (dst, b) in enumerate(((out_r, "bor"), (out_i, "boi"))):
                for c in range(DC):
                    nc.scalar.activation(dst[:, c, :], tmp[:, di * DC + c, :],
                                         AF.Identity, bias=bias[b][:, c:c + 1])

    def gate_act(pr, qoff=0, qw=QT, eng=None):
        """C-phase part 1: two-segment alpha-max-beta-min magnitude,
        mag/BMAG = max(mx/BMAG, (AMAG/BMAG)*mx + mn) with
        mx = max(|out_r|,|out_i|), mn = min(...). The 1/BMAG factor is
        folded into the gate weights on the host. Max relative error ~2.1%;
        the gate's sigmoid sensitivity shrinks that to ~3e-3 on the output.
        Columns [qoff, qoff+qw). eng overrides the engine (drain uses the
        idle GpSimd so the DVE queue stays off the critical path)."""
        eng = eng or nc.vector
        qt, bh = pr["qt"], pr["bh"]
        ar = pr["ar"][:, :, qoff:qoff + qw]
        ai = pr["ai"][:, :, qoff:qoff + qw]
        if "mag" not in pr:
            pr["mag"] = smallpool.tile([P, DC, QT], fp16, tag="mag",
                                       name="mag", bufs=2)
            pr["mx"] = smallpool.tile([P, DC, QT], fp16, tag="mx", name="mx")
            pr["mn"] = smallpool.tile([P, DC, QT], fp16, tag="mn", name="mn")
        mag = pr["mag"][:, :, qoff:qoff + qw]
        mx = pr["mx"][:, :, qoff:qoff + qw]
        mn = pr["mn"][:, :, qoff:qoff + qw]
        ALU = mybir.AluOpType
        with nc.named_scope(f"mag{bh}_{qt}"):
            eng.tensor_tensor(mx[:], ar[:], ai[:], ALU.max)
            eng.tensor_tensor(mn[:], ar[:], ai[:], ALU.min)
            eng.scalar_tensor_tensor(mn[:], mx[:], AMAG / BMAG, mn[:],
                                     op0=ALU.mult, op1=ALU.add)
            eng.scalar_tensor_tensor(mag[:], mx[:], 1.0 / BMAG, mn[:],
                                     op0=ALU.mult, op1=ALU.max)

    def gate_rest(pr, qoff=0, qw=QT, dma2=None):
        """C-phase part 2: gate matmul -> sigmoid -> gated fp16 store, for
        columns [qoff, qoff+qw). sigmoid(z+bg) = 1/(1 + exp(-z-bg)): exp on
        ACT, the +1 also on ACT (Identity, written back into the free PSUM
        tile) so the DVE only carries the reciprocal and the gated muls.
        dma2: second DMA queue for the imag store so the tail's descriptor
        issues parallelize."""
        dma2 = dma2 or nc.sync
        qt, bh = pr["qt"], pr["bh"]
        out_r = pr["out_r"][:, :, qoff:qoff + qw]
        out_i = pr["out_i"][:, :, qoff:qoff + qw]
        mag = pr["mag"][:, :, qoff:qoff + qw]
        with nc.named_scope(f"gate{bh}_{qt}"):
            gate = smallpool.tile([P, DC, QT], f32, tag="gate", name="gate")[:, :, :qw]
            eg = smallpool.tile([P, DC, QT], fp16, tag="eg", name="eg")[:, :, :qw]
            gtiles = []
            for go in range(DC):
                gps = ps_mm.tile([P, QT], f32, tag="mm", name="gps")[:, :qw]
                nc.tensor.matmul(gps, w["gwT"][:, 0, go * P:(go + 1) * P],
                                 mag[:, 0, :], start=True, stop=False)
                nc.tensor.matmul(gps, w["gwT"][:, 1, go * P:(go + 1) * P],
                                 mag[:, 1, :], start=False, stop=True)
                nc.scalar.activation(eg[:, go, :], gps, AF.Exp, scale=-1.0,
                                     bias=bias["bgn"][:, go:go + 1])
                gtiles.append(gps)
            for go in range(DC):
                nc.scalar.activation(gtiles[go][:], eg[:, go, :], AF.Identity,
                                     bias=1.0)
                nc.vector.reciprocal_approx_fast(gate[:, go, :], gtiles[go][:])

            gr = smallpool.tile([P, DC, QT], fp16, tag="goutr", name="goutr",
                                bufs=2)[:, :, :qw]
            gi = smallpool.tile([P, DC, QT], fp16, tag="gouti", name="gouti",
                                bufs=2)[:, :, :qw]
            nc.vector.tensor_mul(gr[:], out_r[:], gate[:])
            nc.vector.tensor_mul(gi[:], out_i[:], gate[:])
            osl = slice(qt * QT + qoff, qt * QT + qoff + qw)
            for c in range(DC):
                nc.sync.dma_start(tens["yr"][bh, c, :, osl], gr[:, c, :])
                dma2.dma_start(tens["yi"][bh, c, :, osl], gi[:, c, :])



    # ---- software-pipelined emission ---------------------------------------
    # steady state per cycle: produce(t) | B(t-1) [+ sums(t)] | C(t-2),
    # with the NEXT head's veff Karatsuba groups woven into the slots
    pending = None       # produced, awaiting B
    pending_gate = None  # B done, awaiting C
    lds = {0: load_inputs(0, first=True)}
    vts = {0: enqueue_veff(0, lds[0])}
    for bh in range(BH_PER_CORE):
        ld = lds[bh]
        if bh > 0:
            while side_q:        # stragglers not placed in slots must be
                side_q.pop(0)()  # emitted before this head's consume phase
        u_r, u_i = proj_u(bh, ld)
        if bh == 0:
            while side_q:        # bh0's veff groups go right after its u
                side_q.pop(0)()  # (PE is DMA-bound here anyway)
        v_r, v_i = vts[bh]
        st = {"bh": bh, "u_r": u_r, "u_i": u_i, "v_r": v_r, "v_i": v_i,
              "xkr": ld["xkr"], "xki": ld["xki"]}
        for qt in range(NQT):
            if bh == 0 and qt == 1:
                continue  # produced during warm-up below
            pr = produce(st, qt)
            if pending is None:
                # warm-up: emit the next produce before the first sums
                # finalization so the reduce-chain latency hides behind PE work
                pr2 = produce(st, 1)
                sums_fin(pr)
                consume_out(pr, then_pr=pr2)
                gate_act(pr)
                pending, pending_gate = pr2, pr
                continue
            if (qt == (2 if bh == 0 else 1)) and bh + 1 < BH_PER_CORE:
                lds[bh + 1] = load_inputs(bh + 1)  # prefetch next (b,h)
                vts[bh + 1] = enqueue_veff(bh + 1, lds[bh + 1])
            consume_out(pending, then_pr=pr)
            gate_act(pending)     # mag chain on DVE right after the B-phase
            gate_rest(pending_gate)
            pending_gate, pending = pending, pr
    # drain the pipeline. pending_gate's mag was already computed in-loop,
    # so its gate matmuls issue first with no wait; the PE's in-order queue
    # then gets all remaining context matmuls ahead of the last gate
    # matmuls; imag stores go out on the scalar HWDGE queue in parallel
    # with sync.
    HQ = QT // 2
    QQ = QT // 4
    consume_out(pending, qoff=0, qw=HQ)      # ctx h0 (PE)
    gate_act(pending, qoff=0, qw=HQ)
    gate_rest(pending_gate, dma2=nc.scalar)  # t-1 gate: mag was computed
    # in-loop, so its MMs are ready the moment the PE reaches them; its
    # DVE work now queues AFTER the h0 norms that free ps_out banks
    consume_out(pending, qoff=HQ, qw=HQ)     # ctx h1 (PE)
    gate_act(pending, qoff=HQ, qw=QQ)        # h1 mag first in the DVE queue,
    gate_act(pending, qoff=HQ + QQ, qw=QQ)   # quartered: q2 unblocks sooner
    gate_rest(pending, qoff=0, qw=HQ, dma2=nc.scalar)
    gate_rest(pending, qoff=HQ, qw=QQ, dma2=nc.scalar)   # last half in
    gate_rest(pending, qoff=HQ + QQ, qw=QQ, dma2=nc.scalar)  # quarters


def _build():
    global _BUILT
    if _BUILT is not None:
        return _BUILT
    from contextlib import ExitStack
    import concourse.tile as tile
    from concourse import bacc, mybir

    f32 = mybir.dt.float32
    fp16 = mybir.dt.float16

    nc = bacc.Bacc("TRN2", target_bir_lowering=False, debug=False,
                   num_devices=NCORES)

    tens = {}
    for name in ("xq_r", "xq_i", "xkv_r", "xkv_i"):
        tens[name] = nc.dram_tensor(name, [BH_PER_CORE, P, DC, T], fp16,
                                    kind="ExternalInput").ap()
    tens["wpack"] = nc.dram_tensor("wpack", [P, DC, NW * D], fp16,
                                   kind="ExternalInput").ap()
    tens["bpack"] = nc.dram_tensor("bpack", [P, NB * DC], f32,
                                   kind="ExternalInput").ap()
    for name in ("yr", "yi"):
        tens[name] = nc.dram_tensor(name, [BH_PER_CORE, DC, P, T], fp16,
                                    kind="ExternalOutput").ap()

    with tile.TileContext(nc) as tc:
        with ExitStack() as ctx:
            _emit_kernel(nc, tc, ctx, tens)

    nc.compile()
    _BUILT = nc
    return nc


def _lhsT_pack(m):
    """[din, dout] stationary weight -> [P, DC, D] fp16 (din chunked,
    partition-major so the DMA is contiguous)."""
    return np.ascontiguousarray(
        m.reshape(DC, P, D).transpose(1, 0, 2).astype(F16))


def _bias_pack(b):
    """[D] bias -> [P, DC] f32 (per-partition d-major layout)."""
    return np.ascontiguousarray(np.asarray(b).reshape(DC, P).T.astype(np.float32))


def _x_pack(x):
    """[BH, T, D] fp32 -> [BH, P, DC, T] fp16 (d-major, partition-major so
    the DMA is contiguous per partition)."""
    xb = x.astype(F16)
    return np.ascontiguousarray(xb.reshape(BH, T, DC, P).transpose(0, 3, 2, 1))


def kernel(**inputs):
    inputs = {k: np.asarray(v) for k, v in inputs.items()}

    nc = _build()
    from concourse.bass_utils import run_bass_kernel_spmd

    xq_r = _x_pack(inputs["q_in_r"].reshape(BH, T, D))
    xq_i = _x_pack(inputs["q_in_i"].reshape(BH, T, D))
    xkv_r = _x_pack(inputs["kv_in_r"].reshape(BH, T, D))
    xkv_i = _x_pack(inputs["kv_in_i"].reshape(BH, T, D))

    f64 = np.float64
    qwr = inputs["q_wr"].astype(f64); qwi = inputs["q_wi"].astype(f64)
    kwr = inputs["k_wr"].astype(f64); kwi = inputs["k_wi"].astype(f64)
    vwr = inputs["v_wr"].astype(f64); vwi = inputs["v_wi"].astype(f64)
    owr = inputs["o_wr"].astype(f64); owi = inputs["o_wi"].astype(f64)
    gw = inputs["gate_w"].astype(f64)

    # u-projection: scores^T = (xq@A + xq_i@Bm ...) against raw kv inputs
    A = qwr.T @ kwr + qwi.T @ kwi
    Bm = qwr.T @ kwi - qwi.T @ kwr
    # veff: W_eff = W_o W_v (complex product)
    Wer = owr @ vwr - owi @ vwi
    Wei = owr @ vwi + owi @ vwr

    # gate weights pre-scaled by BMAG: the kernel's mag tile carries
    # max(mx, AMAG*mx + BMAG*mn) / BMAG
    wmats = {"A": A, "Bm": Bm, "Bmn": -Bm,
             "WerT": Wer.T, "WeiT": Wei.T, "WpT": Wer.T + Wei.T,
             "gwT": (BMAG * gw).T}
    wpack = np.concatenate([_lhsT_pack(wmats[n]) for n in WNAMES], axis=-1)

    # u bias rows (b_q folded through the k weights; b_k drops: it only adds
    # per-q constants to scores, which softmax ignores)
    bqr = inputs["q_br"].astype(f64); bqi = inputs["q_bi"].astype(f64)
    bur = bqr @ kwr + bqi @ kwi
    bui = bqi @ kwr - bqr @ kwi
    # out bias: b_out = W_o b_v + b_o (v bias survives softmax row-sums = 1)
    vbr = inputs["v_br"].astype(f64); vbi = inputs["v_bi"].astype(f64)
    bor = inputs["o_br"].astype(f64) + owr @ vbr - owi @ vbi
    boi = inputs["o_bi"].astype(f64) + owi @ vbr + owr @ vbi

    bmats = {"bur": bur, "bui": bui, "bor": bor, "boi": boi,
             "bgn": -inputs["gate_b"].astype(f64)}
    bpack = np.concatenate([_bias_pack(bmats[n]) for n in BNAMES], axis=-1)

    consts = {"wpack": np.ascontiguousarray(wpack),
              "bpack": np.ascontiguousarray(bpack)}

    in_maps = []
    for c in range(NCORES):
        sl = slice(c * BH_PER_CORE, (c + 1) * BH_PER_CORE)
        m = dict(consts)
        m["xq_r"] = xq_r[sl]
        m["xq_i"] = xq_i[sl]
        m["xkv_r"] = xkv_r[sl]
        m["xkv_i"] = xkv_i[sl]
        in_maps.append(m)

    res = run_bass_kernel_spmd(nc, in_maps, core_ids=list(range(NCORES)))

    def unpack(name):
        full = np.concatenate([res.results[c][name] for c in range(NCORES)], axis=0)
        # [BH, DC, P, T] -> [BH, T, DC*P] -> [B, H, T, D]
        return np.ascontiguousarray(
            full.transpose(0, 3, 1, 2).reshape(B, H, T, D).astype(np.float32))

    return unpack("yr"), unpack("yi")


if __name__ == "__main__":
    # smoke test with random inputs
    rng = np.random.default_rng(0)
    fake = {}
    for nm in ("q_in_r", "q_in_i", "kv_in_r", "kv_in_i"):
        fake[nm] = rng.standard_normal((B, H, T, D), dtype=np.float32)
    for p in ("q", "k", "v", "o"):
        fake[f"{p}_wr"] = rng.standard_normal((D, D), dtype=np.float32) * 0.044
        fake[f"{p}_wi"] = rng.standard_normal((D, D), dtype=np.float32) * 0.044
        fake[f"{p}_br"] = np.zeros(D, np.float32)
        fake[f"{p}_bi"] = np.zeros(D, np.float32)
    fake["gate_w"] = rng.standard_normal((D, D), dtype=np.float32) * 0.044
    fake["gate_b"] = np.zeros(D, np.float32)
    yr, yi = kernel(**fake)
    print("OK", yr.shape, yi.shape, yr.dtype)

